# revision 1
# baseline (speedup 1.0000x reference)
"""Trainium2 Bass kernel for nn_EventProcessor (ragged events -> per-slot MLP).

Contract: kernel(**inputs) takes the FULL unsharded inputs and returns the
FULL [B, 4096] float32 output. Internally the batch slots (and their events)
are sharded by batch_idx range across 8 NeuronCores; the small MLP weights are
replicated (data-parallel, per the sharding hint).

Per core the device computes, for its 2048 slots:
  1. segment max of event confidence per slot (events pre-binned [slot, K] on
     host, padded with -1) + first-event-attaining-max bin position
  2. indirect-DMA gather of the winning event's feature row
  3. a one-hot/feature transpose so layer 1 becomes a single K=10 matmul:
     h.T = relu(L1.T @ [onehot(6); conf; lx; ly; valid])  with
     L1 = [ (W1[:, :1024] @ type_emb.T).T ; scaled W1 misc cols ; b1 ]
     (the type-embedding matmul is input-independent weight folding)
  4. out[slot] = h @ W2.T (+ b2), zeroed for empty slots
All matmuls run as float32r (full PE rate, ~1e-4 matmul error).
"""

import numpy as np

P = 128          # partitions
M_CORES = 8
B_FULL = 16384
E_FULL = 131072
B_LOC = B_FULL // M_CORES      # 2048 slots per core
G = B_LOC // P                 # 16 slot groups per core
N_TYPES = 6
D_IN = 1027                    # 1024 emb + conf + 2 loc
HID = 2048
D_OUT = 4096
K2 = HID // P                  # 16 contraction tiles in layer 2
NW = 256                       # layer-2 output chunk width
NCH = D_OUT // NW
KL1 = N_TYPES + 3 + 1          # layer-1 contraction: onehot + misc + valid

_CACHE: dict = {}


def _build(K: int, use_b2: bool, stage: str = "full"):
    import concourse.bacc as bacc
    import concourse.bass as bass
    import concourse.mybir as mybir
    import concourse.tile as tile
    from concourse.masks import make_identity

    f32 = mybir.dt.float32
    f32r = mybir.dt.float32r
    i32 = mybir.dt.int32
    Alu = mybir.AluOpType

    nc = bacc.Bacc("TRN2", target_bir_lowering=False, debug=True)

    conf_d = nc.dram_tensor("conf", [P, G * K], f32, kind="ExternalInput")
    feat_d = nc.dram_tensor("featrows", [B_LOC * K, 5], f32, kind="ExternalInput")
    l1_d = nc.dram_tensor("l1w", [KL1, HID], f32r, kind="ExternalInput")
    # host pre-permuted: w2l[p, n*K2*NW + k*NW + w] = W2.T[k*P + p, n*NW + w]
    w2_d = nc.dram_tensor("w2t", [P, NCH * K2 * NW], f32r, kind="ExternalInput")
    if use_b2:
        b2_d = nc.dram_tensor("b2r", [1, D_OUT], f32, kind="ExternalInput")
    out_d = nc.dram_tensor("out", [B_LOC, D_OUT], f32, kind="ExternalOutput")
    if stage != "full":
        dbg_d = nc.dram_tensor("dbg", [P, 4096], f32, kind="ExternalOutput")

    BIG = 1e9

    with tile.TileContext(nc) as tc:
        with (
            tc.tile_pool(name="cpool", bufs=1) as cpool,
            tc.tile_pool(name="work", bufs=2) as work,
            tc.tile_pool(name="wpool", bufs=2) as wpool,
            tc.tile_pool(name="opool", bufs=4) as opool,
            tc.tile_pool(name="pst", bufs=1, space="PSUM") as pst,
            tc.tile_pool(name="psl1", bufs=2, space="PSUM") as psl1,
            tc.tile_pool(name="psdum", bufs=1, space="PSUM") as psdum,
            tc.tile_pool(name="psl2", bufs=4, space="PSUM") as psl2,
        ):
            # ---- constants ----
            ident = cpool.tile([P, P], f32)
            make_identity(nc, ident[:])

            iota6_i = cpool.tile([P, N_TYPES], i32)
            nc.gpsimd.iota(iota6_i[:], pattern=[[1, N_TYPES]], channel_multiplier=0)
            iota6_f = cpool.tile([P, N_TYPES], f32)
            nc.vector.tensor_copy(out=iota6_f[:], in_=iota6_i[:])

            iotaK_i = cpool.tile([P, G * K], i32)
            nc.gpsimd.iota(iotaK_i[:], pattern=[[0, G], [1, K]], channel_multiplier=0)
            iotaK_f = cpool.tile([P, G * K], f32)
            nc.vector.tensor_copy(out=iotaK_f[:], in_=iotaK_i[:])

            offbase = cpool.tile([P, G], i32)
            nc.gpsimd.iota(offbase[:], pattern=[[P * K, G]], channel_multiplier=K)

            l1_sb = cpool.tile([KL1, HID], f32r)
            nc.sync.dma_start(out=l1_sb[:], in_=l1_d[:])

            if use_b2:
                b2_sb = cpool.tile([1, D_OUT], f32)
                nc.sync.dma_start(out=b2_sb[:], in_=b2_d[:])

            # ---- segment max / argmax over binned confidences ----
            conf_sb = work.tile([P, G * K], f32, tag="conf")
            nc.sync.dma_start(out=conf_sb[:], in_=conf_d[:])
            conf3 = conf_sb[:].rearrange("p (g k) -> p g k", k=K)

            # HAM warm-up / keep-warm: the PE clock gate (HAM) halves the PE
            # clock after ~3.4us of low PE activity and has been seen to stay
            # throttled for 100us+ once tripped. The front-end and layer-1
            # phases are latency-bound with sparse PE duty, so we fill every
            # PE pipeline gap with dependency-free dummy matmuls into a
            # dedicated PSUM bank that nothing reads. They consume only
            # otherwise-idle PE cycles but keep the activity monitor happy.
            dum = psdum.tile([P, 64], f32)

            def pe_filler(n):
                if stage == "front":
                    return
                for _ in range(n):
                    nc.tensor.matmul(
                        out=dum[:],
                        lhsT=ident[:],
                        rhs=conf_sb[:, 0:64],
                        start=True,
                        stop=True,
                        skip_group_check=True,
                    )

            pe_filler(100)

            segmax = work.tile([P, G], f32, tag="segmax")
            nc.vector.tensor_reduce(
                out=segmax[:], in_=conf3, axis=mybir.AxisListType.X, op=Alu.max
            )
            has01 = cpool.tile([P, G], f32)
            nc.vector.tensor_scalar(
                out=has01[:], in0=segmax[:], scalar1=0.0, scalar2=None, op0=Alu.is_ge
            )

            cand = work.tile([P, G * K], f32, tag="cand")
            nc.vector.tensor_tensor(
                out=cand[:].rearrange("p (g k) -> p g k", k=K),
                in0=conf3,
                in1=segmax[:].unsqueeze(2).to_broadcast([P, G, K]),
                op=Alu.is_equal,
            )
            nc.vector.tensor_scalar(
                out=cand[:], in0=cand[:], scalar1=-BIG, scalar2=BIG,
                op0=Alu.mult, op1=Alu.add,
            )
            nc.vector.tensor_tensor(
                out=cand[:], in0=cand[:], in1=iotaK_f[:], op=Alu.add
            )
            pstar = work.tile([P, G], f32, tag="pstar")
            nc.vector.tensor_reduce(
                out=pstar[:],
                in_=cand[:].rearrange("p (g k) -> p g k", k=K),
                axis=mybir.AxisListType.X,
                op=Alu.min,
            )
            offs = work.tile([P, G], i32, tag="offs")
            nc.vector.tensor_copy(out=offs[:], in_=pstar[:])
            nc.vector.tensor_tensor(
                out=offs[:], in0=offs[:], in1=offbase[:], op=Alu.add
            )

            # ---- gather winning rows (one batched indirect DMA), build
            # transposed layer-1 rhs ----
            rhs1 = cpool.tile([KL1, B_LOC], f32r)
            if stage != "front":
                h_sb = cpool.tile([P, K2 * B_LOC], f32r)
            SC = B_LOC // 512  # slot chunks of 512

            def g_block(g):
                feat_g = work.tile([P, 5], f32, tag="feat")
                nc.gpsimd.indirect_dma_start(
                    out=feat_g[:],
                    out_offset=None,
                    in_=feat_d[:],
                    in_offset=bass.IndirectOffsetOnAxis(
                        ap=offs[:, g : g + 1], axis=0
                    ),
                )
                trans_in = work.tile([P, KL1], f32, tag="ti")
                nc.vector.tensor_tensor(
                    out=trans_in[:, 0:N_TYPES],
                    in0=feat_g[:, 0:1].to_broadcast([P, N_TYPES]),
                    in1=iota6_f[:],
                    op=Alu.is_equal,
                )
                nc.vector.tensor_copy(
                    out=trans_in[:, N_TYPES : N_TYPES + 4], in_=feat_g[:, 1:5]
                )
                tp = pst.tile([KL1, P], f32, tag="tp")
                nc.tensor.transpose(out=tp[:], in_=trans_in[:], identity=ident[:])
                pe_filler(6)
                nc.scalar.copy(out=rhs1[:, g * P : (g + 1) * P], in_=tp[:])

            def l1_block(s):
                for m in range(K2):
                    ph = psl1.tile([P, 512], f32, tag="ph")
                    nc.tensor.matmul(
                        out=ph[:],
                        lhsT=l1_sb[:, m * P : (m + 1) * P],
                        rhs=rhs1[:, s * 512 : (s + 1) * 512],
                        start=True,
                        stop=True,
                    )
                    pe_filler(3)
                    h_out = h_sb[:, m * B_LOC + s * 512 : m * B_LOC + (s + 1) * 512]
                    if m % 4 == 3:
                        nc.vector.tensor_scalar_max(h_out, ph[:], 0.0)
                    else:
                        nc.scalar.activation(
                            out=h_out,
                            in_=ph[:],
                            func=mybir.ActivationFunctionType.Relu,
                        )

            if stage == "front":
                for g in range(G):
                    g_block(g)
                dbg_sb = cpool.tile([P, B_LOC + G], f32)
                nc.vector.tensor_copy(out=dbg_sb[0:KL1, 0:B_LOC], in_=rhs1[:])
                nc.vector.tensor_copy(out=dbg_sb[:, B_LOC : B_LOC + G], in_=has01[:])
                nc.sync.dma_start(out=dbg_d[:, 0 : B_LOC + G], in_=dbg_sb[:])

            # ---- interleaved gather/transpose blocks + layer-1 s-blocks ----
            if stage != "front":
                for s in range(SC):
                    for g in range(4 * s, 4 * s + 4):
                        g_block(g)
                    l1_block(s)
            if stage == "l1":
                dbg_sb2 = cpool.tile([P, B_LOC], f32)
                nc.vector.tensor_copy(out=dbg_sb2[:], in_=h_sb[:, 0:B_LOC])
                nc.sync.dma_start(out=dbg_d[:, 0:B_LOC], in_=dbg_sb2[:])

            # ---- layer 2: out[slot, n] = h @ W2T (+ b2), masked by has01 ----
            if stage.startswith("full") or stage.startswith("l2"):
                n_chunks = 1 if stage == "l2one" else NCH
                CW = K2 * NW
                for n in range(n_chunks):
                    w2_sb = wpool.tile([P, CW], f32r, tag="w2")
                    nc.sync.dma_start(
                        out=w2_sb[:], in_=w2_d[:, n * CW : (n + 1) * CW]
                    )
                    for m in range(G):
                        po = psl2.tile([P, NW], f32, tag="po")
                        for k in range(K2):
                            nc.tensor.matmul(
                                out=po[:],
                                lhsT=h_sb[:, k * B_LOC + m * P : k * B_LOC + (m + 1) * P],
                                rhs=w2_sb[:, k * NW : (k + 1) * NW],
                                start=(k == 0),
                                stop=(k == K2 - 1),
                            )
                        ob = opool.tile([P, NW], f32, tag="ob")
                        if use_b2:
                            nc.vector.tensor_tensor(
                                out=po[:],
                                in0=po[:],
                                in1=b2_sb[0:1, n * NW : (n + 1) * NW].partition_broadcast(P),
                                op=Alu.add,
                            )
                        if m % 2 == 0:
                            nc.vector.tensor_scalar_mul(
                                ob[:], po[:], has01[:, m : m + 1]
                            )
                        else:
                            nc.scalar.activation(
                                out=ob[:],
                                in_=po[:],
                                func=mybir.ActivationFunctionType.Copy,
                                scale=has01[:, m : m + 1],
                            )
                        nc.sync.dma_start(
                            out=out_d[m * P : (m + 1) * P, n * NW : (n + 1) * NW],
                            in_=ob[:],
                        )

    nc.compile()
    return nc


def _prep(event_type, confidence, location, batch_idx, type_emb, W1, b1, W2, b2):
    """Host-side sharding/binning + input-independent weight folding."""
    E = confidence.shape[0]
    B = B_FULL

    counts = np.bincount(batch_idx, minlength=B)
    K = int(counts.max())
    K = max(8, -(-K // 8) * 8)

    starts = np.zeros(B + 1, np.int64)
    np.cumsum(counts, out=starts[1:])
    order = np.argsort(batch_idx, kind="stable")
    sorted_slot = batch_idx[order]
    pos = np.arange(E, dtype=np.int64) - starts[sorted_slot]
    flat = sorted_slot * K + pos

    conf_bins = np.full(B * K, -1.0, np.float32)
    conf_bins[flat] = confidence[order]
    conf_bins = conf_bins.reshape(B, K)

    featrows = np.zeros((B * K, 5), np.float32)
    featrows[flat, 0] = event_type[order].astype(np.float32)
    featrows[flat, 1] = confidence[order]
    featrows[flat, 2] = location[order, 0]
    featrows[flat, 3] = location[order, 1]
    featrows[flat, 4] = 1.0

    A = (W1[:, :1024].astype(np.float64) @ type_emb.astype(np.float64).T)
    Wmisc = W1[:, 1024:1027].astype(np.float64) / np.array([1.0, 640.0, 480.0])
    l1w = np.concatenate(
        [A.T, Wmisc.T, b1[None, :].astype(np.float64)], axis=0
    ).astype(np.float32)
    w2t = W2.T.astype(np.float32)  # [HID, D_OUT]
    w2l = np.ascontiguousarray(
        w2t.reshape(K2, P, NCH, NW).transpose(1, 2, 0, 3).reshape(P, NCH * K2 * NW)
    )

    in_maps = []
    for c in range(M_CORES):
        sl = slice(c * B_LOC, (c + 1) * B_LOC)
        conf_dev = np.ascontiguousarray(
            conf_bins[sl].reshape(G, P, K).transpose(1, 0, 2).reshape(P, G * K)
        )
        m = {
            "conf": conf_dev,
            "featrows": featrows[c * B_LOC * K : (c + 1) * B_LOC * K],
            "l1w": l1w,
            "w2t": w2l,
        }
        if np.any(b2):
            m["b2r"] = np.ascontiguousarray(b2[None, :].astype(np.float32))
        in_maps.append(m)
    return K, in_maps


def kernel(
    event_type,
    confidence,
    location,
    batch_idx,
    batch_size,
    type_emb,
    W1,
    b1,
    W2,
    b2,
    _trace=False,
):
    from concourse.bass_utils import run_bass_kernel_spmd

    event_type = np.asarray(event_type)
    confidence = np.asarray(confidence, dtype=np.float32)
    location = np.asarray(location, dtype=np.float32)
    batch_idx = np.asarray(batch_idx)
    type_emb = np.asarray(type_emb, dtype=np.float32)
    W1 = np.asarray(W1, dtype=np.float32)
    b1 = np.asarray(b1, dtype=np.float32)
    W2 = np.asarray(W2, dtype=np.float32)
    b2 = np.asarray(b2, dtype=np.float32)
    B = int(batch_size)
    assert B == B_FULL and confidence.shape[0] == E_FULL
    assert W1.shape == (HID, D_IN) and W2.shape == (D_OUT, HID)

    K, in_maps = _prep(
        event_type, confidence, location, batch_idx, type_emb, W1, b1, W2, b2
    )
    use_b2 = bool(np.any(b2))

    key = (K, use_b2)
    if key not in _CACHE:
        _CACHE[key] = _build(K, use_b2)
    nc = _CACHE[key]

    res = run_bass_kernel_spmd(
        nc, in_maps, core_ids=list(range(M_CORES)), trace=_trace
    )
    out = np.concatenate([r["out"] for r in res.results], axis=0)
    if _trace:
        kernel.last_result = res
    return out

